# revision 25
# baseline (speedup 1.0000x reference)
"""AGCN (2-layer GCN) forward on 8 TRN2 NeuronCores.

Math (reference):
    agg(h)[d] = sum_{(s,d) in E+selfloops} dinv[s]*dinv[d] * h[s]
    out = relu(agg(relu(agg(x) @ W1 + b1) @ W2) + b2)

Distribution: nodes are dealt (serpentine, by in-degree) onto 8 cores x 49
blocks of 128, balancing per-(core,block) edge counts across cores (the SPMD
program pads each block to the max over cores). Per dst block the normalized
adjacency slice is a host-built stack of selection matrices S.T
[128 slots, 128 dst] (slot = distinct source; entry = summed edge norm).
Aggregation = dma_gather of source rows (bf16, SWDGE, 4 parallel queues) +
TensorE matmuls accumulating in f32 PSUM -- no scatter.

Both x and z live in the same permuted layout (row = core*span + local), so
one idx array pair serves both layers. The layout is split at local row 3200
(blocks 0-24 | 25-48) giving two gather bases of < 32768 rows each (int16
reach), and z is all-gathered in two chunks: chunk A (blocks 0-24) is issued
mid-phase-1 and overlaps the remaining phase-1 work; phase-2's part-A
gathers then overlap chunk B's collective.

Self-loops use a contiguous DMA of the block's own rows (no gather). Layer 1
computes s1.T feature-major (lhsT=gathered, rhs=S.T), chains W1/W2 in
feature-major, transposes via TensorE, stores z bf16 node-major; layer 2
aggregates with the same S tiles (lhsT=S.T, rhs=gathered), node-major out.
"""
import sys

for _p in ("/opt/trn_rl_repo", "/root/.axon_site/_ro/trn_rl_repo"):
    if _p not in sys.path:
        sys.path.append(_p)

import ml_dtypes
import numpy as np

from concourse import bacc, mybir, tile
from concourse.bass_utils import run_bass_kernel_spmd

BF16 = ml_dtypes.bfloat16
FP8 = ml_dtypes.float8_e4m3

N = 50000
D = 128
H = 512
NCORES = 8
R = N // NCORES          # 6250 real rows per core
NBLK = (R + 127) // 128  # 49
RPAD = NBLK * 128        # 6272
GROUPS = [(0, 9), (9, 8), (17, 8), (25, 8), (33, 8), (41, 4), (45, 2), (47, 2)]
NGRP = len(GROUPS)
BLK_A = 25               # blocks 0..24 -> part a; 25..48 -> part b
HLOC = BLK_A * 128       # 3200
HB = RPAD - HLOC         # 3072
OP_IDX = 1024            # max indices per dma_gather (SWDGE ring limit)
PARTS = ("a", "b")
AG_A_AFTER_GROUP = 3     # issue all-gather A after this group (covers blk<=27)


def _deal(ids_sorted, cores):
    """Serpentine-deal sorted node ids over (core, block) bins."""
    bins = [(c, b) for c in cores for b in range(NBLK)]
    caps = {(c, b): (128 if b < NBLK - 1 else R - 128 * (NBLK - 1)) for c, b in bins}
    fill = {bin_: 0 for bin_ in bins}
    n2c = {}
    n2l = {}
    it = iter(ids_sorted)
    done = False
    for r in range(128):
        order = bins if r % 2 == 0 else bins[::-1]
        for bin_ in order:
            if fill[bin_] < caps[bin_]:
                try:
                    n = next(it)
                except StopIteration:
                    done = True
                    break
                c, b = bin_
                n2c[n] = c
                n2l[n] = b * 128 + fill[bin_]
                fill[bin_] += 1
        if done:
            break
    return n2c, n2l


GSTART = np.array([g0 for g0, _ in GROUPS])


def _prep(x, edge_index, W1, b1, W2, b2):
    src = np.asarray(edge_index[0], dtype=np.int64)
    dst = np.asarray(edge_index[1], dtype=np.int64)
    indeg = np.bincount(dst, minlength=N)
    deg = indeg.astype(np.float64) + 1.0  # + self loop
    dinv = 1.0 / np.sqrt(deg)
    norm = np.ones(src.shape[0], np.float32)  # split-norm: S carries counts

    n2c, n2l = _deal(np.argsort(-indeg, kind="stable"), list(range(NCORES)))
    node2core = np.empty(N, np.int64)
    node2local = np.empty(N, np.int64)
    node2core[list(n2c.keys())] = list(n2c.values())
    node2local[list(n2l.keys())] = list(n2l.values())
    nodes_at = np.full((NCORES, RPAD), -1, np.int64)
    nodes_at[node2core, node2local] = np.arange(N)

    core = node2core[dst]
    eloc = node2local[dst]
    blk = eloc // 128
    dcol = eloc % 128
    sl = node2local[src]
    is_a = sl < HLOC

    percore = []
    cnts = np.zeros((2, NCORES, NBLK), np.int64)
    for c in range(NCORES):
        m = core == c
        info = {}
        for pi, part in enumerate(PARTS):
            hm = m & (is_a if part == "a" else ~is_a)
            key = blk[hm] * N + src[hm]
            uk, inv = np.unique(key, return_inverse=True)
            ub, us = uk // N, uk % N
            order = np.argsort(ub, kind="stable")
            sb = ub[order]
            slot = np.zeros(uk.shape[0], np.int64)
            if sb.size:
                start = np.r_[True, sb[1:] != sb[:-1]]
                grp_start = np.maximum.accumulate(
                    np.where(start, np.arange(sb.size), 0)
                )
                slot[order] = np.arange(sb.size) - grp_start
            usc, usl = node2core[us], node2local[us]
            ur = np.where(usl < HLOC, usc * HLOC + usl, usc * HB + (usl - HLOC))
            info[part] = dict(ub=ub, ur=ur, slot=slot, inv=inv,
                              dcol=dcol[hm], nrm=norm[hm])
            cnts[pi, c] = np.bincount(ub, minlength=NBLK)
        percore.append(info)

    NSP = {part: -(-cnts[pi].max(axis=0) // 128) for pi, part in enumerate(PARTS)}
    NS = NSP["a"] + NSP["b"] + 1
    sub_off = np.r_[0, np.cumsum(NS)]
    TOT_SUB = int(sub_off[-1])
    part_sub0 = {"a": np.zeros(NBLK, np.int64), "b": NSP["a"].copy()}

    def stream_meta(ns_half):
        offs, lens = [], []
        for g0, gn in GROUPS:
            o = 0
            for b in range(g0, g0 + gn):
                offs.append(o)
                o += int(ns_half[b]) * 128
            lens.append(o)
        return np.array(offs), lens

    boff, glen, ops = {}, {}, {}
    for part in PARTS:
        boff[part], glen[part] = stream_meta(NSP[part])
        ops[part] = [
            [OP_IDX] * (L // OP_IDX) + ([L % OP_IDX] if L % OP_IDX else [])
            for L in glen[part]
        ]

    meta = dict(NSP=NSP, NS=NS, sub_off=sub_off, TOT_SUB=TOT_SUB,
                part_sub0=part_sub0, boff=boff, glen=glen, ops=ops)

    w1_bf = np.asarray(W1, np.float32).astype(BF16)
    w2_bf = np.asarray(W2, np.float32).astype(BF16)
    b1_f = np.asarray(b1, np.float32).reshape(4, 128).T.copy()
    b2_bc = np.tile(np.asarray(b2, np.float32)[None, :], (128, 1))
    eye = np.eye(128, dtype=BF16)
    xs_bf = (np.asarray(x, np.float32) * dinv[:, None].astype(np.float32)).astype(BF16)

    def wrap(idx):
        k = idx.shape[0]
        w = idx.reshape(k // 16, 16).T.astype(np.int16)
        return np.ascontiguousarray(np.tile(w, (8, 1)))

    glen_pref = {p: np.r_[0, np.cumsum(glen[p])] for p in PARTS}
    L = {p: int(glen_pref[p][-1]) for p in PARTS}

    dvb = np.zeros((NBLK, 128, 128), dtype=BF16)
    dvc = np.zeros((NCORES, 128, NBLK), dtype=np.float32)
    # (filled per core below)
    # permuted x copies (gather bases, same layout as the z all-gather)
    xp_a = np.zeros((NCORES * HLOC, D), dtype=BF16)
    xp_b = np.zeros((NCORES * HB, D), dtype=BF16)
    for c in range(NCORES):
        va = nodes_at[c, :HLOC] >= 0
        xp_a[c * HLOC : (c + 1) * HLOC][va] = xs_bf[nodes_at[c, :HLOC][va]]
        vb = nodes_at[c, HLOC:] >= 0
        xp_b[c * HB : (c + 1) * HB][vb] = xs_bf[nodes_at[c, HLOC:][vb]]

    in_maps = []
    for c in range(NCORES):
        info = percore[c]
        st = np.zeros((128, TOT_SUB, 128), dtype=np.float32)
        idxs = {}
        for part in PARTS:
            d = info[part]
            i1 = np.zeros(L[part], np.int64)
            if d["ub"].size:
                gid = np.searchsorted(GSTART, d["ub"], side="right") - 1
                spos = glen_pref[part][gid] + boff[part][d["ub"]] + d["slot"]
                i1[spos] = d["ur"]
                esub = (sub_off[d["ub"]] + part_sub0[part][d["ub"]] + d["slot"] // 128)[d["inv"]]
                epart = (d["slot"] % 128)[d["inv"]]
                np.add.at(st, (epart, esub, d["dcol"]), d["nrm"])
            idxs[part] = wrap(i1)
        for b in range(NBLK):
            s_idx = sub_off[b] + NS[b] - 1
            nvalid = min(128, R - b * 128)
            orig = nodes_at[c, b * 128 : b * 128 + nvalid]
            st[np.arange(nvalid), s_idx, np.arange(nvalid)] = 1.0

        x_own = np.zeros((RPAD, D), dtype=BF16)
        validn = nodes_at[c] >= 0
        x_own[validn] = xs_bf[nodes_at[c][validn]]
        dvb_c = np.zeros((NBLK, 128, 128), dtype=BF16)
        dvc_c = np.zeros((128, NBLK), dtype=np.float32)
        for b in range(NBLK):
            nv = min(128, R - b * 128)
            dv = dinv[nodes_at[c, b * 128 : b * 128 + nv]].astype(np.float32)
            dvb_c[b, :, :nv] = np.tile(dv[None, :], (128, 1)).astype(BF16)
            dvc_c[:nv, b] = dv

        in_maps.append(
            {
                "xp_a": xp_a,
                "xp_b": xp_b,
                "x_own": x_own,
                "st": st.astype(FP8),
                "dvb": dvb_c,
                "dvc": dvc_c,
                "ia": idxs["a"],
                "ib": idxs["b"],
                "w1": w1_bf,
                "w2": w2_bf,
                "b1": b1_f,
                "b2bc": b2_bc,
                "eye": eye,
            }
        )
    return in_maps, meta, dict(nodes_at=nodes_at)


def build(meta):
    NSP, NS = meta["NSP"], meta["NS"]
    sub_off, TOT_SUB = meta["sub_off"], meta["TOT_SUB"]
    boff, glen, ops = meta["boff"], meta["glen"], meta["ops"]
    L = {p: sum(glen[p]) for p in PARTS}

    nc = bacc.Bacc("TRN2", target_bir_lowering=False, debug=False,
                   num_devices=NCORES, num_swdge_queues=4)
    f32, bf16, i16 = mybir.dt.float32, mybir.dt.bfloat16, mybir.dt.int16

    xp_a = nc.declare_dram_parameter("xp_a", [NCORES * HLOC, D], bf16, isOutput=False)
    xp_b = nc.declare_dram_parameter("xp_b", [NCORES * HB, D], bf16, isOutput=False)
    x_own = nc.declare_dram_parameter("x_own", [RPAD, D], bf16, isOutput=False)
    fp8 = mybir.dt.float8e4
    st_d = nc.declare_dram_parameter("st", [128, TOT_SUB, 128], fp8, isOutput=False)
    dvb_d = nc.declare_dram_parameter("dvb", [NBLK, 128, 128], bf16, isOutput=False)
    dvc_d = nc.declare_dram_parameter("dvc", [128, NBLK], f32, isOutput=False)
    ia_d = nc.declare_dram_parameter("ia", [128, L["a"] // 16], i16, isOutput=False)
    ib_d = nc.declare_dram_parameter("ib", [128, L["b"] // 16], i16, isOutput=False)
    w1_d = nc.declare_dram_parameter("w1", [D, H], bf16, isOutput=False)
    w2_d = nc.declare_dram_parameter("w2", [H, D], bf16, isOutput=False)
    b1_d = nc.declare_dram_parameter("b1", [128, 4], f32, isOutput=False)
    b2_d = nc.declare_dram_parameter("b2bc", [128, 128], f32, isOutput=False)
    eye_d = nc.declare_dram_parameter("eye", [128, 128], bf16, isOutput=False)
    out_d = nc.declare_dram_parameter("out", [RPAD, D], f32, isOutput=True)

    z_own_a = nc.dram_tensor("z_own_a", [HLOC, D], bf16)
    z_own_b = nc.dram_tensor("z_own_b", [HB, D], bf16)
    zf_a = nc.dram_tensor("zf_a", [NCORES * HLOC, D], bf16, addr_space="Shared")
    zf_b = nc.dram_tensor("zf_b", [NCORES * HB, D], bf16, addr_space="Shared")

    MAXSUB = {p: max(1, max(glen[p]) // 128) for p in PARTS}

    with tile.TileContext(nc) as tc:
        with (
            tc.tile_pool(name="const", bufs=1) as cpool,
            tc.tile_pool(name="ga", bufs=4) as gapool,
            tc.tile_pool(name="gb", bufs=4) as gbpool,
            tc.tile_pool(name="stp", bufs=3) as stpool,
            tc.tile_pool(name="small", bufs=3) as spool,
            tc.tile_pool(name="psA", bufs=2, space="PSUM") as psA,
            tc.tile_pool(name="psB", bufs=2, space="PSUM") as psB,
        ):
            w1_t = cpool.tile([128, H], bf16)
            nc.sync.dma_start(out=w1_t[:], in_=w1_d[:])
            w2_t = cpool.tile([128, 4, 128], bf16)
            nc.sync.dma_start(out=w2_t[:], in_=w2_d[:].rearrange("(m p) o -> p m o", p=128))
            b1_t = cpool.tile([128, 4], f32)
            nc.sync.dma_start(out=b1_t[:], in_=b1_d[:])
            b2_t = cpool.tile([128, 128], f32)
            nc.sync.dma_start(out=b2_t[:], in_=b2_d[:])
            eye_t = cpool.tile([128, 128], bf16)
            nc.sync.dma_start(out=eye_t[:], in_=eye_d[:])
            dvc_t = cpool.tile([128, NBLK], f32)
            nc.sync.dma_start(out=dvc_t[:], in_=dvc_d[:])
            # resident idx tiles, loaded once, shared by both layers
            ia_t = cpool.tile([128, L["a"] // 16], i16)
            nc.sync.dma_start(out=ia_t[:], in_=ia_d[:])
            ib_t = cpool.tile([128, L["b"] // 16], i16)
            nc.sync.dma_start(out=ib_t[:], in_=ib_d[:])
            idx_t = {"a": ia_t, "b": ib_t}

            qn = [0]
            ag_a_done = [False]

            def gather_stream(g, part, src_ap, dst_tile):
                Lg = glen[part][g]
                if Lg == 0:
                    return
                cum = sum(glen[part][:g])
                it = idx_t[part]
                o = 0
                for sz in ops[part][g]:
                    nc.gpsimd.dma_gather(
                        dst_tile[:, o // 128 : (o + sz) // 128, :], src_ap,
                        it[:, (cum + o) // 16 : (cum + o + sz) // 16], sz, sz, D,
                        queue_num=qn[0] % 4,
                    )
                    qn[0] += 1
                    o += sz

            def own_rows(layer, b):
                if layer == 1:
                    return x_own[b * 128 : (b + 1) * 128, :]
                if b < BLK_A:
                    return z_own_a[b * 128 : (b + 1) * 128, :]
                return z_own_b[(b - BLK_A) * 128 : (b - BLK_A + 1) * 128, :]

            def do_blocks(g, layer, gtiles):
                g0, gn = GROUPS[g]
                for br in range(gn):
                    b = g0 + br
                    ns = int(NS[b])
                    nvalid = min(128, R - b * 128)
                    xo = spool.tile([128, 128], bf16, tag="xo")
                    nc.sync.dma_start(out=xo[:], in_=own_rows(layer, b))
                    st_t = stpool.tile([128, int(NS.max()), 128], fp8, tag="st")
                    nc.sync.dma_start(
                        out=st_t[:, :ns, :],
                        in_=st_d[:, sub_off[b] : sub_off[b] + ns, :],
                    )
                    p = psA.tile([128, 128], f32, tag="p1")
                    for s in range(ns):
                        if s == ns - 1:
                            g_sl = xo[:]
                        elif s < int(NSP["a"][b]):
                            g_sl = gtiles["a"][:, boff["a"][b] // 128 + s, :]
                        else:
                            g_sl = gtiles["b"][:, boff["b"][b] // 128 + (s - int(NSP["a"][b])), :]
                        if layer == 1:
                            nc.tensor.matmul(
                                p[:], g_sl, st_t[:, s, :],
                                start=(s == 0), stop=(s == ns - 1),
                            )
                        else:
                            nc.tensor.matmul(
                                p[:], st_t[:, s, :], g_sl,
                                start=(s == 0), stop=(s == ns - 1),
                            )
                    if layer == 1:
                        dvb_t = spool.tile([128, 128], bf16, tag="dvb")
                        nc.sync.dma_start(out=dvb_t[:], in_=dvb_d[b])
                        at = spool.tile([128, 128], bf16, tag="at")
                        nc.vector.tensor_tensor(
                            at[:], p[:], dvb_t[:], mybir.AluOpType.mult
                        )
                        hs = spool.tile([128, 4, 128], bf16, tag="hs")
                        for mi in range(4):
                            hp = psB.tile([128, 128], f32, tag="hp")
                            nc.tensor.matmul(
                                hp[:], w1_t[:, mi * 128 : (mi + 1) * 128], at[:],
                                start=True, stop=True,
                            )
                            nc.scalar.activation(
                                hs[:, mi, :], hp[:],
                                mybir.ActivationFunctionType.Relu,
                                bias=b1_t[:, mi : mi + 1],
                            )
                        zp = psA.tile([128, 128], f32, tag="zp")
                        for mi in range(4):
                            nc.tensor.matmul(
                                zp[:], w2_t[:, mi, :], hs[:, mi, :],
                                start=(mi == 0), stop=(mi == 3),
                            )
                        zs = spool.tile([128, 128], bf16, tag="zs")
                        nc.vector.tensor_copy(zs[:], zp[:])
                        ztp = psB.tile([128, 128], bf16, tag="ztp")
                        nc.tensor.transpose(ztp[:], zs[:], eye_t[:])
                        zts = spool.tile([128, 128], bf16, tag="zts")
                        nc.vector.tensor_scalar_mul(
                            zts[:], ztp[:], dvc_t[:, b : b + 1]
                        )
                        nc.sync.dma_start(out=own_rows(2, b), in_=zts[:])
                    else:
                        ob = spool.tile([128, 128], f32, tag="ob")
                        nc.vector.scalar_tensor_tensor(
                            ob[:], p[:], dvc_t[:, b : b + 1], b2_t[:],
                            mybir.AluOpType.mult, mybir.AluOpType.add,
                        )
                        o2 = spool.tile([128, 128], f32, tag="o2")
                        nc.scalar.activation(
                            o2[:], ob[:], mybir.ActivationFunctionType.Relu
                        )
                        nc.sync.dma_start(
                            out=out_d[b * 128 : b * 128 + nvalid, :],
                            in_=o2[:nvalid, :],
                        )

            def ag(zo, zf):
                nc.gpsimd.collective_compute(
                    "AllGather",
                    mybir.AluOpType.bypass,
                    replica_groups=[list(range(NCORES))],
                    ins=[zo[:].opt()],
                    outs=[zf[:].opt()],
                )

            # ---------------- phase 1 ----------------
            for g in range(NGRP):
                gt = {
                    "a": gapool.tile([128, MAXSUB["a"], 128], bf16, tag="ga", name="ga"),
                    "b": gbpool.tile([128, MAXSUB["b"], 128], bf16, tag="gb", name="gb"),
                }
                gather_stream(g, "a", xp_a[:], gt["a"])
                gather_stream(g, "b", xp_b[:], gt["b"])
                do_blocks(g, 1, gt)
                if not ag_a_done[0] and GROUPS[g][0] + GROUPS[g][1] >= BLK_A:
                    ag(z_own_a, zf_a)  # blocks 0..24 done; overlaps the rest
                    ag_a_done[0] = True

            ag(z_own_b, zf_b)

            # ---------------- phase 2 ----------------
            # part-a gathers depend only on zf_a, so Tile can run the first
            # few during AG-b's flight
            for g in range(NGRP):
                gt = {
                    "a": gapool.tile([128, MAXSUB["a"], 128], bf16, tag="ga", name="ga"),
                    "b": gbpool.tile([128, MAXSUB["b"], 128], bf16, tag="gb", name="gb"),
                }
                gather_stream(g, "a", zf_a[:], gt["a"])
                gather_stream(g, "b", zf_b[:], gt["b"])
                do_blocks(g, 2, gt)

    nc.compile()
    return nc


_CACHE = {}


def kernel(x, edge_index, W1, b1, W2, b2):
    in_maps, meta, perm = _prep(x, edge_index, W1, b1, W2, b2)
    key = (tuple(meta["NS"]),) + tuple(tuple(meta["glen"][p]) for p in PARTS)
    if key not in _CACHE:
        _CACHE[key] = build(meta)
    nc = _CACHE[key]
    res = run_bass_kernel_spmd(nc, in_maps, core_ids=list(range(NCORES)))
    nodes_at = perm["nodes_at"]
    out = np.empty((N, D), np.float32)
    for c in range(NCORES):
        validn = nodes_at[c] >= 0
        out[nodes_at[c][validn]] = np.asarray(res.results[c]["out"], np.float32)[validn]
    return out


# revision 26
# speedup vs baseline: 1.0206x; 1.0206x over previous
"""AGCN (2-layer GCN) forward on 8 TRN2 NeuronCores.

Math (reference):
    agg(h)[d] = sum_{(s,d) in E+selfloops} dinv[s]*dinv[d] * h[s]
    out = relu(agg(relu(agg(x) @ W1 + b1) @ W2) + b2)

Distribution: nodes are dealt (serpentine, by in-degree) onto 8 cores x 49
blocks of 128, balancing per-(core,block) edge counts across cores (the SPMD
program pads each block to the max over cores). Per dst block the normalized
adjacency slice is a host-built stack of selection matrices S.T
[128 slots, 128 dst] (slot = distinct source; entry = summed edge norm).
Aggregation = dma_gather of source rows (bf16, SWDGE, 4 parallel queues) +
TensorE matmuls accumulating in f32 PSUM -- no scatter.

Both x and z live in the same permuted layout (row = core*span + local), so
one idx array pair serves both layers. The layout is split at local row 3200
(blocks 0-24 | 25-48) giving two gather bases of < 32768 rows each (int16
reach), and z is all-gathered in two chunks: chunk A (blocks 0-24) is issued
mid-phase-1 and overlaps the remaining phase-1 work; phase-2's part-A
gathers then overlap chunk B's collective.

Self-loops use a contiguous DMA of the block's own rows (no gather). Layer 1
computes s1.T feature-major (lhsT=gathered, rhs=S.T), chains W1/W2 in
feature-major, transposes via TensorE, stores z bf16 node-major; layer 2
aggregates with the same S tiles (lhsT=S.T, rhs=gathered), node-major out.
"""
import sys

for _p in ("/opt/trn_rl_repo", "/root/.axon_site/_ro/trn_rl_repo"):
    if _p not in sys.path:
        sys.path.append(_p)

import ml_dtypes
import numpy as np

from concourse import bacc, mybir, tile
from concourse.bass_utils import run_bass_kernel_spmd

BF16 = ml_dtypes.bfloat16
FP8 = ml_dtypes.float8_e4m3

N = 50000
D = 128
H = 512
NCORES = 8
R = N // NCORES          # 6250 real rows per core
NBLK = (R + 127) // 128  # 49
RPAD = NBLK * 128        # 6272
GROUPS = [(0, 9), (9, 8), (17, 8), (25, 8), (33, 8), (41, 4), (45, 4)]
NGRP = len(GROUPS)
BLK_A = 31               # blocks 0..30 -> part a; 31..48 -> part b
HLOC = BLK_A * 128       # 3200
HB = RPAD - HLOC         # 3072
OP_IDX = 1024            # max indices per dma_gather (SWDGE ring limit)
PARTS = ("a", "b")
AG_A_AFTER_GROUP = 3     # issue all-gather A after this group (covers blk<=27)


def _deal(ids_sorted, cores):
    """Serpentine-deal sorted node ids over (core, block) bins."""
    bins = [(c, b) for c in cores for b in range(NBLK)]
    caps = {(c, b): (128 if b < NBLK - 1 else R - 128 * (NBLK - 1)) for c, b in bins}
    fill = {bin_: 0 for bin_ in bins}
    n2c = {}
    n2l = {}
    it = iter(ids_sorted)
    done = False
    for r in range(128):
        order = bins if r % 2 == 0 else bins[::-1]
        for bin_ in order:
            if fill[bin_] < caps[bin_]:
                try:
                    n = next(it)
                except StopIteration:
                    done = True
                    break
                c, b = bin_
                n2c[n] = c
                n2l[n] = b * 128 + fill[bin_]
                fill[bin_] += 1
        if done:
            break
    return n2c, n2l


GSTART = np.array([g0 for g0, _ in GROUPS])


def _prep(x, edge_index, W1, b1, W2, b2):
    src = np.asarray(edge_index[0], dtype=np.int64)
    dst = np.asarray(edge_index[1], dtype=np.int64)
    indeg = np.bincount(dst, minlength=N)
    deg = indeg.astype(np.float64) + 1.0  # + self loop
    dinv = 1.0 / np.sqrt(deg)
    norm = np.ones(src.shape[0], np.float32)  # split-norm: S carries counts

    n2c, n2l = _deal(np.argsort(-indeg, kind="stable"), list(range(NCORES)))
    node2core = np.empty(N, np.int64)
    node2local = np.empty(N, np.int64)
    node2core[list(n2c.keys())] = list(n2c.values())
    node2local[list(n2l.keys())] = list(n2l.values())
    nodes_at = np.full((NCORES, RPAD), -1, np.int64)
    nodes_at[node2core, node2local] = np.arange(N)

    core = node2core[dst]
    eloc = node2local[dst]
    blk = eloc // 128
    dcol = eloc % 128
    sl = node2local[src]
    is_a = sl < HLOC

    percore = []
    cnts = np.zeros((2, NCORES, NBLK), np.int64)
    for c in range(NCORES):
        m = core == c
        info = {}
        for pi, part in enumerate(PARTS):
            hm = m & (is_a if part == "a" else ~is_a)
            key = blk[hm] * N + src[hm]
            uk, inv = np.unique(key, return_inverse=True)
            ub, us = uk // N, uk % N
            order = np.argsort(ub, kind="stable")
            sb = ub[order]
            slot = np.zeros(uk.shape[0], np.int64)
            if sb.size:
                start = np.r_[True, sb[1:] != sb[:-1]]
                grp_start = np.maximum.accumulate(
                    np.where(start, np.arange(sb.size), 0)
                )
                slot[order] = np.arange(sb.size) - grp_start
            usc, usl = node2core[us], node2local[us]
            ur = np.where(usl < HLOC, usc * HLOC + usl, usc * HB + (usl - HLOC))
            info[part] = dict(ub=ub, ur=ur, slot=slot, inv=inv,
                              dcol=dcol[hm], nrm=norm[hm])
            cnts[pi, c] = np.bincount(ub, minlength=NBLK)
        percore.append(info)

    NSP = {part: -(-cnts[pi].max(axis=0) // 128) for pi, part in enumerate(PARTS)}
    NS = NSP["a"] + NSP["b"] + 1
    sub_off = np.r_[0, np.cumsum(NS)]
    TOT_SUB = int(sub_off[-1])
    part_sub0 = {"a": np.zeros(NBLK, np.int64), "b": NSP["a"].copy()}

    def stream_meta(ns_half):
        offs, lens = [], []
        for g0, gn in GROUPS:
            o = 0
            for b in range(g0, g0 + gn):
                offs.append(o)
                o += int(ns_half[b]) * 128
            lens.append(o)
        return np.array(offs), lens

    boff, glen, ops = {}, {}, {}
    for part in PARTS:
        boff[part], glen[part] = stream_meta(NSP[part])
        ops[part] = [
            [OP_IDX] * (L // OP_IDX) + ([L % OP_IDX] if L % OP_IDX else [])
            for L in glen[part]
        ]

    meta = dict(NSP=NSP, NS=NS, sub_off=sub_off, TOT_SUB=TOT_SUB,
                part_sub0=part_sub0, boff=boff, glen=glen, ops=ops)

    w1_bf = np.asarray(W1, np.float32).astype(BF16)
    w2_bf = np.asarray(W2, np.float32).astype(BF16)
    b1_f = np.asarray(b1, np.float32).reshape(4, 128).T.copy()
    b2_bc = np.tile(np.asarray(b2, np.float32)[None, :], (128, 1))
    eye = np.eye(128, dtype=BF16)
    xs_bf = (np.asarray(x, np.float32) * dinv[:, None].astype(np.float32)).astype(BF16)

    def wrap(idx):
        k = idx.shape[0]
        w = idx.reshape(k // 16, 16).T.astype(np.int16)
        return np.ascontiguousarray(np.tile(w, (8, 1)))

    glen_pref = {p: np.r_[0, np.cumsum(glen[p])] for p in PARTS}
    L = {p: int(glen_pref[p][-1]) for p in PARTS}

    dvb = np.zeros((NBLK, 128, 128), dtype=BF16)
    dvc = np.zeros((NCORES, 128, NBLK), dtype=np.float32)
    # (filled per core below)
    # permuted x copies (gather bases, same layout as the z all-gather)
    xp_a = np.zeros((NCORES * HLOC, D), dtype=BF16)
    xp_b = np.zeros((NCORES * HB, D), dtype=BF16)
    for c in range(NCORES):
        va = nodes_at[c, :HLOC] >= 0
        xp_a[c * HLOC : (c + 1) * HLOC][va] = xs_bf[nodes_at[c, :HLOC][va]]
        vb = nodes_at[c, HLOC:] >= 0
        xp_b[c * HB : (c + 1) * HB][vb] = xs_bf[nodes_at[c, HLOC:][vb]]

    in_maps = []
    for c in range(NCORES):
        info = percore[c]
        st = np.zeros((128, TOT_SUB, 128), dtype=np.float32)
        idxs = {}
        for part in PARTS:
            d = info[part]
            i1 = np.zeros(L[part], np.int64)
            if d["ub"].size:
                gid = np.searchsorted(GSTART, d["ub"], side="right") - 1
                spos = glen_pref[part][gid] + boff[part][d["ub"]] + d["slot"]
                i1[spos] = d["ur"]
                esub = (sub_off[d["ub"]] + part_sub0[part][d["ub"]] + d["slot"] // 128)[d["inv"]]
                epart = (d["slot"] % 128)[d["inv"]]
                np.add.at(st, (epart, esub, d["dcol"]), d["nrm"])
            idxs[part] = wrap(i1)
        for b in range(NBLK):
            s_idx = sub_off[b] + NS[b] - 1
            nvalid = min(128, R - b * 128)
            orig = nodes_at[c, b * 128 : b * 128 + nvalid]
            st[np.arange(nvalid), s_idx, np.arange(nvalid)] = 1.0

        x_own = np.zeros((RPAD, D), dtype=BF16)
        validn = nodes_at[c] >= 0
        x_own[validn] = xs_bf[nodes_at[c][validn]]
        dvb_c = np.zeros((NBLK, 128, 128), dtype=BF16)
        dvc_c = np.zeros((128, NBLK), dtype=np.float32)
        for b in range(NBLK):
            nv = min(128, R - b * 128)
            dv = dinv[nodes_at[c, b * 128 : b * 128 + nv]].astype(np.float32)
            dvb_c[b, :, :nv] = np.tile(dv[None, :], (128, 1)).astype(BF16)
            dvc_c[:nv, b] = dv

        in_maps.append(
            {
                "xp_a": xp_a,
                "xp_b": xp_b,
                "x_own": x_own,
                "st": st.astype(FP8),
                "dvb": dvb_c,
                "dvc": dvc_c,
                "ia": idxs["a"],
                "ib": idxs["b"],
                "w1": w1_bf,
                "w2": w2_bf,
                "b1": b1_f,
                "b2bc": b2_bc,
                "eye": eye,
            }
        )
    return in_maps, meta, dict(nodes_at=nodes_at)


def build(meta):
    NSP, NS = meta["NSP"], meta["NS"]
    sub_off, TOT_SUB = meta["sub_off"], meta["TOT_SUB"]
    boff, glen, ops = meta["boff"], meta["glen"], meta["ops"]
    L = {p: sum(glen[p]) for p in PARTS}

    nc = bacc.Bacc("TRN2", target_bir_lowering=False, debug=False,
                   num_devices=NCORES, num_swdge_queues=4)
    f32, bf16, i16 = mybir.dt.float32, mybir.dt.bfloat16, mybir.dt.int16

    xp_a = nc.declare_dram_parameter("xp_a", [NCORES * HLOC, D], bf16, isOutput=False)
    xp_b = nc.declare_dram_parameter("xp_b", [NCORES * HB, D], bf16, isOutput=False)
    x_own = nc.declare_dram_parameter("x_own", [RPAD, D], bf16, isOutput=False)
    fp8 = mybir.dt.float8e4
    st_d = nc.declare_dram_parameter("st", [128, TOT_SUB, 128], fp8, isOutput=False)
    dvb_d = nc.declare_dram_parameter("dvb", [NBLK, 128, 128], bf16, isOutput=False)
    dvc_d = nc.declare_dram_parameter("dvc", [128, NBLK], f32, isOutput=False)
    ia_d = nc.declare_dram_parameter("ia", [128, L["a"] // 16], i16, isOutput=False)
    ib_d = nc.declare_dram_parameter("ib", [128, L["b"] // 16], i16, isOutput=False)
    w1_d = nc.declare_dram_parameter("w1", [D, H], bf16, isOutput=False)
    w2_d = nc.declare_dram_parameter("w2", [H, D], bf16, isOutput=False)
    b1_d = nc.declare_dram_parameter("b1", [128, 4], f32, isOutput=False)
    b2_d = nc.declare_dram_parameter("b2bc", [128, 128], f32, isOutput=False)
    eye_d = nc.declare_dram_parameter("eye", [128, 128], bf16, isOutput=False)
    out_d = nc.declare_dram_parameter("out", [RPAD, D], f32, isOutput=True)

    z_own_a = nc.dram_tensor("z_own_a", [HLOC, D], bf16)
    z_own_b = nc.dram_tensor("z_own_b", [HB, D], bf16)
    zf_a = nc.dram_tensor("zf_a", [NCORES * HLOC, D], bf16, addr_space="Shared")
    zf_b = nc.dram_tensor("zf_b", [NCORES * HB, D], bf16, addr_space="Shared")

    MAXSUB = {p: max(1, max(glen[p]) // 128) for p in PARTS}

    with tile.TileContext(nc) as tc:
        with (
            tc.tile_pool(name="const", bufs=1) as cpool,
            tc.tile_pool(name="ga", bufs=4) as gapool,
            tc.tile_pool(name="gb", bufs=4) as gbpool,
            tc.tile_pool(name="stp", bufs=3) as stpool,
            tc.tile_pool(name="small", bufs=3) as spool,
            tc.tile_pool(name="psA", bufs=2, space="PSUM") as psA,
            tc.tile_pool(name="psB", bufs=2, space="PSUM") as psB,
        ):
            w1_t = cpool.tile([128, H], bf16)
            nc.sync.dma_start(out=w1_t[:], in_=w1_d[:])
            w2_t = cpool.tile([128, 4, 128], bf16)
            nc.sync.dma_start(out=w2_t[:], in_=w2_d[:].rearrange("(m p) o -> p m o", p=128))
            b1_t = cpool.tile([128, 4], f32)
            nc.sync.dma_start(out=b1_t[:], in_=b1_d[:])
            b2_t = cpool.tile([128, 128], f32)
            nc.sync.dma_start(out=b2_t[:], in_=b2_d[:])
            eye_t = cpool.tile([128, 128], bf16)
            nc.sync.dma_start(out=eye_t[:], in_=eye_d[:])
            dvc_t = cpool.tile([128, NBLK], f32)
            nc.sync.dma_start(out=dvc_t[:], in_=dvc_d[:])
            # resident idx tiles, loaded once, shared by both layers
            ia_t = cpool.tile([128, L["a"] // 16], i16)
            nc.sync.dma_start(out=ia_t[:], in_=ia_d[:])
            ib_t = cpool.tile([128, L["b"] // 16], i16)
            nc.sync.dma_start(out=ib_t[:], in_=ib_d[:])
            idx_t = {"a": ia_t, "b": ib_t}

            qn = [0]
            ag_a_done = [False]

            def gather_stream(g, part, src_ap, dst_tile):
                Lg = glen[part][g]
                if Lg == 0:
                    return
                cum = sum(glen[part][:g])
                it = idx_t[part]
                o = 0
                for sz in ops[part][g]:
                    nc.gpsimd.dma_gather(
                        dst_tile[:, o // 128 : (o + sz) // 128, :], src_ap,
                        it[:, (cum + o) // 16 : (cum + o + sz) // 16], sz, sz, D,
                        queue_num=qn[0] % 4,
                    )
                    qn[0] += 1
                    o += sz

            def own_rows(layer, b):
                if layer == 1:
                    return x_own[b * 128 : (b + 1) * 128, :]
                if b < BLK_A:
                    return z_own_a[b * 128 : (b + 1) * 128, :]
                return z_own_b[(b - BLK_A) * 128 : (b - BLK_A + 1) * 128, :]

            def do_blocks(g, layer, gtiles):
                g0, gn = GROUPS[g]
                for br in range(gn):
                    b = g0 + br
                    ns = int(NS[b])
                    nvalid = min(128, R - b * 128)
                    xo = spool.tile([128, 128], bf16, tag="xo")
                    nc.sync.dma_start(out=xo[:], in_=own_rows(layer, b))
                    st_t = stpool.tile([128, int(NS.max()), 128], fp8, tag="st")
                    nc.sync.dma_start(
                        out=st_t[:, :ns, :],
                        in_=st_d[:, sub_off[b] : sub_off[b] + ns, :],
                    )
                    p = psA.tile([128, 128], f32, tag="p1")
                    for s in range(ns):
                        if s == ns - 1:
                            g_sl = xo[:]
                        elif s < int(NSP["a"][b]):
                            g_sl = gtiles["a"][:, boff["a"][b] // 128 + s, :]
                        else:
                            g_sl = gtiles["b"][:, boff["b"][b] // 128 + (s - int(NSP["a"][b])), :]
                        if layer == 1:
                            nc.tensor.matmul(
                                p[:], g_sl, st_t[:, s, :],
                                start=(s == 0), stop=(s == ns - 1),
                            )
                        else:
                            nc.tensor.matmul(
                                p[:], st_t[:, s, :], g_sl,
                                start=(s == 0), stop=(s == ns - 1),
                            )
                    if layer == 1:
                        dvb_t = spool.tile([128, 128], bf16, tag="dvb")
                        nc.sync.dma_start(out=dvb_t[:], in_=dvb_d[b])
                        at = spool.tile([128, 128], bf16, tag="at")
                        nc.vector.tensor_tensor(
                            at[:], p[:], dvb_t[:], mybir.AluOpType.mult
                        )
                        hs = spool.tile([128, 4, 128], bf16, tag="hs")
                        for mi in range(4):
                            hp = psB.tile([128, 128], f32, tag="hp")
                            nc.tensor.matmul(
                                hp[:], w1_t[:, mi * 128 : (mi + 1) * 128], at[:],
                                start=True, stop=True,
                            )
                            nc.scalar.activation(
                                hs[:, mi, :], hp[:],
                                mybir.ActivationFunctionType.Relu,
                                bias=b1_t[:, mi : mi + 1],
                            )
                        zp = psA.tile([128, 128], f32, tag="zp")
                        for mi in range(4):
                            nc.tensor.matmul(
                                zp[:], w2_t[:, mi, :], hs[:, mi, :],
                                start=(mi == 0), stop=(mi == 3),
                            )
                        zs = spool.tile([128, 128], bf16, tag="zs")
                        nc.vector.tensor_copy(zs[:], zp[:])
                        ztp = psB.tile([128, 128], bf16, tag="ztp")
                        nc.tensor.transpose(ztp[:], zs[:], eye_t[:])
                        zts = spool.tile([128, 128], bf16, tag="zts")
                        nc.vector.tensor_scalar_mul(
                            zts[:], ztp[:], dvc_t[:, b : b + 1]
                        )
                        nc.sync.dma_start(out=own_rows(2, b), in_=zts[:])
                    else:
                        ob = spool.tile([128, 128], f32, tag="ob")
                        nc.vector.scalar_tensor_tensor(
                            ob[:], p[:], dvc_t[:, b : b + 1], b2_t[:],
                            mybir.AluOpType.mult, mybir.AluOpType.add,
                        )
                        o2 = spool.tile([128, 128], f32, tag="o2")
                        nc.scalar.activation(
                            o2[:], ob[:], mybir.ActivationFunctionType.Relu
                        )
                        nc.sync.dma_start(
                            out=out_d[b * 128 : b * 128 + nvalid, :],
                            in_=o2[:nvalid, :],
                        )

            def ag(zo, zf):
                nc.gpsimd.collective_compute(
                    "AllGather",
                    mybir.AluOpType.bypass,
                    replica_groups=[list(range(NCORES))],
                    ins=[zo[:].opt()],
                    outs=[zf[:].opt()],
                )

            # ---------------- phase 1 ----------------
            for g in range(NGRP):
                gt = {
                    "a": gapool.tile([128, MAXSUB["a"], 128], bf16, tag="ga", name="ga"),
                    "b": gbpool.tile([128, MAXSUB["b"], 128], bf16, tag="gb", name="gb"),
                }
                gather_stream(g, "a", xp_a[:], gt["a"])
                gather_stream(g, "b", xp_b[:], gt["b"])
                do_blocks(g, 1, gt)
                if not ag_a_done[0] and GROUPS[g][0] + GROUPS[g][1] >= BLK_A:
                    ag(z_own_a, zf_a)  # blocks 0..24 done; overlaps the rest
                    ag_a_done[0] = True

            ag(z_own_b, zf_b)

            # ---------------- phase 2 ----------------
            # part-a gathers depend only on zf_a, so Tile can run the first
            # few during AG-b's flight
            for g in range(NGRP):
                gt = {
                    "a": gapool.tile([128, MAXSUB["a"], 128], bf16, tag="ga", name="ga"),
                    "b": gbpool.tile([128, MAXSUB["b"], 128], bf16, tag="gb", name="gb"),
                }
                gather_stream(g, "a", zf_a[:], gt["a"])
                gather_stream(g, "b", zf_b[:], gt["b"])
                do_blocks(g, 2, gt)

    nc.compile()
    return nc


_CACHE = {}


def kernel(x, edge_index, W1, b1, W2, b2):
    in_maps, meta, perm = _prep(x, edge_index, W1, b1, W2, b2)
    key = (tuple(meta["NS"]),) + tuple(tuple(meta["glen"][p]) for p in PARTS)
    if key not in _CACHE:
        _CACHE[key] = build(meta)
    nc = _CACHE[key]
    res = run_bass_kernel_spmd(nc, in_maps, core_ids=list(range(NCORES)))
    nodes_at = perm["nodes_at"]
    out = np.empty((N, D), np.float32)
    for c in range(NCORES):
        validn = nodes_at[c] >= 0
        out[nodes_at[c][validn]] = np.asarray(res.results[c]["out"], np.float32)[validn]
    return out
